# revision 69
# baseline (speedup 1.0000x reference)
"""CTRGC graph-conv kernel for Trainium2, 8-core data-parallel.

Math (attention branch of the reference is dead code, all BNs fold away):
    y[n,o,t,v] = relu( sum_k sum_w Amix_k[w,v] * H_k[n,o,t,w] + B[o] )
    H_k[n,o,t,w] = sum_c W2_k[o,c] * x[n,c,t,w]

Device plan per core (n = 16 local batches):
  Stage A (conv): x is the *stationary* matmul operand: for each chunk of
    126 = 7t*18w positions, out[(t,w), (k,o)] = x_chunk[c,128].T @ W[c,192].
    This lands H transposed -- (t,w) on partitions -- which is what stage B
    needs.  Chunks 0..17 use x half-1 on partitions 0:64, chunks 18..36 use
    x half-2 on partitions 64:128, so pairs of matmuls run concurrently on
    disjoint PE row-groups.  lhsT slices are 128 wide -- the 2 junk columns
    only pollute out partitions 126/127 which are never read.
  Stage B (mix): stationary blockdiag(Amix_k x7) padded to [128,128] --
    32-aligned weight column counts take a ~2x faster LDWEIGHTS path (95ns
    vs 177-230ns measured for 126 cols); rhs = H slices [128, 8 chunks x
    64]; PSUM-accumulate over k.  Row 126 of H holds the bias pattern (B[o]
    at k=0 column slots, DMA'd per n) and row 126 of blkA_0 is ones => the
    matmul adds B[o] along the free dim for free; row 127 is zero in both
    (zero, not stale SBUF, so 0*0 cannot make NaNs).
  Epilogue: relu split across ScalarE/VectorE, fp16 out (host upcasts).

All device I/O is fp16 (host casts x down and y back up; PE accumulates in
fp32, so the extra error is ~2e-4).  Software-pipelined by one n: mix(n-1)
is emitted after conv(n) so its H inputs are already evacuated when the PE
reaches it.  Strict DVE/ACT alternation of PSUM-evac ops is load-bearing:
consecutive same-engine evacs stall PSUM bank recycling.

Chunking: t-groups of 7; chunk 36 covers t=249..255 (recomputes t=249..251,
host discards the duplicates).  Device output y_dev[n, (tl,v)=126,
(ch,o)=2368]; host reassembles to (n,o,t,v).

Measured on 8x trn2 (axon): HW exec ~145-146 us, rel err (absmax) ~5.7e-4.
Engine balance at final: PE ~116 us matmul union (conv LDW-path-bound at
~94ns/chunk-load; mix stream+drain-bound), DVE/ACT ~100 us each (PSUM evac
at 1 elem/cycle/lane), plus ~14 us head + ~15 us tail of Tile barriers/DMA.
Keep ALL matmul weight-operand column counts 32-aligned (conv lhsT 128,
blkA 128) -- misaligned counts fall off the fast LDWEIGHTS path.
"""

import os
import sys
import types

import numpy as np

import concourse.bass as bass
import concourse.mybir as mybir
import concourse.tile as tile
from concourse import bacc
from concourse.bass_utils import run_bass_kernel_spmd

EPS = 1e-5
N, CIN, T, V = 128, 64, 256, 18
NK = 3
CS = 64  # per-subset output channels
NCORES = 8
NLOC = N // NCORES  # 16
TW = T * V  # 4608
TG = 7  # t positions per chunk
PCH = TG * V  # 126 partitions per chunk
NCH = 37  # 36 aligned chunks + 1 overlap chunk (t0=249)
H1W = 2268  # x half-1 cols: t 0..125   (chunks 0..17,  partitions 0:64)
H2W = 2340  # x half-2 cols: t 126..255 (chunks 18..36, partitions 64:128)
XW = 2344  # x tile width (pad so 128-wide lhsT slices stay in bounds)
KO = NK * CS  # 192
HW = NCH * KO  # 7104 H columns
GRP = 8  # chunks per mix psum group (8*64 = 512 = one PSUM bank)
NGRP = 5  # groups of sizes 8,8,8,8,5
YW = NCH * CS  # 2368 y columns
F16 = mybir.dt.float16
F32 = mybir.dt.float32

LAST_RESULT = None  # BassKernelResults of the most recent run (for test.py)


def _ensure_ntff_hook() -> bool:
    """Register the axon NTFF profile hook if the image's antenv lacks it.

    Dev-only (used when BASS_TRACE=1); returns True if tracing can work.
    """
    try:
        from antenv.axon_hooks import get_axon_ntff_profile_hook

        return get_axon_ntff_profile_hook() is not None
    except ImportError:
        pass
    try:
        import antenv
        from trn_agent_boot.trn_boot import _ntff_profile_via_ctypes

        hook = _ntff_profile_via_ctypes("/opt/axon/libaxon_pjrt.so")
        if hook is None:
            return False
        mod = types.ModuleType("antenv.axon_hooks")
        holder = [hook]
        mod.set_axon_ntff_profile_hook = lambda h: holder.__setitem__(0, h)
        mod.get_axon_ntff_profile_hook = lambda: holder[0]
        sys.modules["antenv.axon_hooks"] = mod
        antenv.axon_hooks = mod
        return True
    except Exception as e:  # pragma: no cover - dev path
        print(f"ntff hook setup failed: {e}", file=sys.stderr)
        return False


def _chunk_col(ch: int) -> int:
    """Column offset of chunk ch inside its x half."""
    if ch < 18:
        return 126 * ch
    if ch < 36:
        return 126 * (ch - 18)
    return 2214  # t0 = 249


def _fold_weights(A, PA, Wta, bta, g_ta, b_ta, m_ta, v_ta, g_bn, b_bn, m_bn, v_bn):
    s1 = g_ta / np.sqrt(v_ta + EPS)  # (3, 64)
    s2 = g_bn / np.sqrt(v_bn + EPS)  # (64,)
    W2 = Wta * s1[:, :, None] * s2[None, :, None]  # (3, 64, 64) [k, o, c]
    b1 = (bta - m_ta) * s1 + b_ta  # (3, 64)
    B = (b1.sum(0) - m_bn) * s2 + b_bn  # (64,)
    Amix = A + PA  # (3, 18, 18)

    # wt [128, 192]: rows = c (duplicated on 64:128 for PE row-group 1),
    # cols = (k, o)
    w_koc = np.ascontiguousarray(W2.transpose(2, 0, 1)).reshape(64, KO)
    wt = np.zeros((128, KO), np.float16)
    wt[0:64] = w_koc
    wt[64:128] = w_koc

    # blk [128, 3*128]: per k a blockdiag of 7 copies of Amix[k] padded to
    # 128 cols (32-aligned weight loads take the fast LDW path); row 126 of
    # the k=0 slice is ones (bias contraction row); row 127 zero (K=128
    # alignment pad, paired with the zeroed H row 127).
    blk = np.zeros((128, NK * 128), np.float16)
    for k in range(NK):
        for j in range(TG):
            blk[18 * j : 18 * j + 18, 128 * k + 18 * j : 128 * k + 18 * j + 18] = Amix[
                k
            ]
    blk[126, 0:126] = 1.0

    # brow [2, 7104]: row 0 = B[o] at the k=0 column slot of every chunk;
    # row 1 = zeros (lands in H row 127 so the K=128 pad row is defined).
    brow = np.zeros((2, HW), np.float16)
    for ch in range(NCH):
        brow[0, KO * ch : KO * ch + CS] = B
    return wt, blk, brow


def _build_bass() -> bass.Bass:
    nc = bacc.Bacc()
    x_d = nc.dram_tensor("x", [NLOC, CIN, TW], F16, kind="ExternalInput")
    wt_d = nc.dram_tensor("wt", [128, KO], F16, kind="ExternalInput")
    blk_d = nc.dram_tensor("blk", [128, NK * 128], F16, kind="ExternalInput")
    brow_d = nc.dram_tensor("brow", [2, HW], F16, kind="ExternalInput")
    y_d = nc.dram_tensor("y", [NLOC, PCH, YW], F16, kind="ExternalOutput")

    # conv psum tiles: [128, 1024] = 2 banks = four chunks at col offsets
    # {0, 192, 512, 704}
    rg0_tiles = [tuple(range(4 * i, 4 * i + 4)) for i in range(4)] + [(16, 17)]
    rg1_tiles = [tuple(range(18 + 4 * i, 22 + 4 * i)) for i in range(4)] + [
        (34, 35),
        (36,),
    ]

    with tile.TileContext(nc) as tc:
        with (
            tc.tile_pool(name="consts", bufs=1) as consts,
            tc.tile_pool(name="xp", bufs=6) as xpool,
            tc.tile_pool(name="hp", bufs=4) as hpool,
            tc.tile_pool(name="yp", bufs=3) as ypool,
            tc.tile_pool(name="cps", bufs=3, space="PSUM") as cpsum,
            tc.tile_pool(name="yps", bufs=2, space="PSUM") as ypsum,
        ):
            ei = 0  # evac engine alternator

            def load_x(n):
                xt = xpool.tile([128, XW], F16, name="xt", tag="xt")
                nc.sync.dma_start(out=xt[0:64, 0:H1W], in_=x_d[n, :, 0:H1W])
                nc.sync.dma_start(out=xt[64:128, 0:H2W], in_=x_d[n, :, H1W:TW])
                return xt

            def emit_conv(n, xt):
                nonlocal ei
                ht = hpool.tile([128, HW], F16, name="ht", tag="ht")
                nc.gpsimd.dma_start(out=ht[126:128, :], in_=brow_d[:])

                # row-group-paired matmuls
                for ti in range(6):
                    pts = []
                    for rg, tiles in ((0, rg0_tiles), (1, rg1_tiles)):
                        if ti >= len(tiles):
                            pts.append(None)
                            continue
                        pts.append(
                            (
                                cpsum.tile([128, 1024], F32, name="cpt", tag="cpt"),
                                tiles[ti],
                                rg,
                            )
                        )
                    for j in range(4):
                        for ent in pts:
                            if ent is None or j >= len(ent[1]):
                                continue
                            pt, chunks, rg = ent
                            ch = chunks[j]
                            r0 = 64 * rg
                            c0 = _chunk_col(ch)
                            pcol = 512 * (j // 2) + 192 * (j % 2)
                            nc.tensor.matmul(
                                out=pt[:, pcol : pcol + 192],
                                lhsT=xt[r0 : r0 + 64, c0 : c0 + 128],
                                rhs=wt_t[r0 : r0 + 64, :],
                                start=True,
                                stop=True,
                            )
                    for ent in pts:
                        if ent is None:
                            continue
                        pt, chunks, rg = ent
                        nch_t = len(chunks)
                        nb = (nch_t + 1) // 2  # banks used
                        fw = 192 * min(nch_t, 2)
                        src = bass.AP(
                            tensor=pt.tensor,
                            offset=pt.offset,
                            ap=[[pt.ap[0][0], 126], [512, nb], [1, fw]],
                        )
                        dst = bass.AP(
                            tensor=ht.tensor,
                            offset=ht.offset + KO * chunks[0],
                            ap=[[ht.ap[0][0], 126], [fw, nb], [1, fw]],
                        )
                        if ei % 2 == 0:
                            nc.vector.tensor_copy(out=dst, in_=src)
                        else:
                            nc.scalar.copy(out=dst, in_=src)
                        ei += 1
                return ht

            def emit_mix(n, ht):
                # k-outer over group-halves: same blkA_k stationary across the
                # groups of a half -> no weight reload between matmuls, and
                # consecutive matmuls hit different PSUM banks (pipelined).
                nonlocal ei
                yt = ypool.tile([128, YW], F16, name="yt", tag="yt")
                hv = ht[0:128].rearrange("p (c f) -> p c f", f=KO)
                for g in range(NGRP):
                    wg = GRP if g < 4 else NCH - 4 * GRP
                    fw = wg * CS
                    pt = ypsum.tile([128, 512], F32, name="ypt", tag="ypt")
                    out = pt[:, 0:fw].rearrange("p (c o) -> p c o", o=CS)
                    for k in range(NK):
                        rhs = hv[:, GRP * g : GRP * g + wg, CS * k : CS * k + CS]
                        nc.tensor.matmul(
                            out=out,
                            lhsT=blk_t[:, 128 * k : 128 * k + 128],
                            rhs=rhs,
                            start=(k == 0),
                            stop=(k == NK - 1),
                        )
                    # split relu across both engines so the psum bank frees
                    # ~2x sooner (mix is gated on psum recycling)
                    h1 = fw // 2
                    nc.vector.tensor_scalar_max(
                        yt[0:126, 512 * g : 512 * g + h1], pt[0:126, 0:h1], 0.0
                    )
                    nc.scalar.activation(
                        yt[0:126, 512 * g + h1 : 512 * g + fw],
                        pt[0:126, h1:fw],
                        mybir.ActivationFunctionType.Relu,
                    )
                nc.gpsimd.dma_start(out=y_d[n], in_=yt[0:126, :])

            # software pipeline by one n: mix(n-1) is emitted after conv(n),
            # so its H inputs are fully evacuated by the time PE reaches it.
            x0 = load_x(0)  # x(0) ahead of consts: it's the longer pole
            wt_t = consts.tile([128, KO], F16)
            nc.sync.dma_start(out=wt_t[:], in_=wt_d[:])
            blk_t = consts.tile([128, NK * 128], F16)
            nc.sync.dma_start(out=blk_t[:], in_=blk_d[:])

            prev = None
            xt_next = x0
            for n in range(NLOC):
                xt = xt_next if xt_next is not None else load_x(n)
                ht = emit_conv(n, xt)
                xt_next = load_x(n + 1) if n + 1 < NLOC else None
                if prev is not None:
                    emit_mix(n - 1, prev)
                prev = ht
            emit_mix(NLOC - 1, prev)
    nc.compile()
    return nc


def _assemble(parts: list[np.ndarray]) -> np.ndarray:
    """parts: NCORES arrays [NLOC, 126, 2368] -> full (N, 64, 256, 18)."""
    y = np.empty((N, CS, T, V), np.float32)
    for i, p in enumerate(parts):
        r = p.astype(np.float32).reshape(NLOC, TG, V, NCH, CS)  # [n, tl, v, ch, o]
        main = r[:, :, :, 0:36, :].transpose(0, 4, 3, 1, 2).reshape(NLOC, CS, 252, V)
        tail = r[:, 3:7, :, 36, :].transpose(0, 3, 1, 2)  # t = 252..255
        sl = slice(i * NLOC, (i + 1) * NLOC)
        y[sl, :, 0:252, :] = main
        y[sl, :, 252:256, :] = tail
    return y


def kernel(
    x,
    A,
    PA,
    Wta,
    bta,
    g_ta,
    b_ta,
    m_ta,
    v_ta,
    Wsa=None,
    bsa=None,
    Wsb=None,
    bsb=None,
    g_bn=None,
    b_bn=None,
    m_bn=None,
    v_bn=None,
):
    global LAST_RESULT
    f = np.asarray
    wt, blk, brow = _fold_weights(
        f(A, dtype=np.float32),
        f(PA, dtype=np.float32),
        f(Wta, dtype=np.float32),
        f(bta, dtype=np.float32),
        f(g_ta, dtype=np.float32),
        f(b_ta, dtype=np.float32),
        f(m_ta, dtype=np.float32),
        f(v_ta, dtype=np.float32),
        f(g_bn, dtype=np.float32),
        f(b_bn, dtype=np.float32),
        f(m_bn, dtype=np.float32),
        f(v_bn, dtype=np.float32),
    )
    xsh = (
        np.asarray(x, dtype=np.float32)
        .reshape(NCORES, NLOC, CIN, TW)
        .astype(np.float16)
    )
    nc = _build_bass()
    in_maps = [
        {"x": np.ascontiguousarray(xsh[i]), "wt": wt, "blk": blk, "brow": brow}
        for i in range(NCORES)
    ]
    if os.environ.get("BASS_LDWOPT"):  # dev experiment: enable walrus ldw-opt
        import concourse.bass_utils as _bu

        if not getattr(_bu, "_ldwopt_patched", False):
            _orig = _bu.run_command

            def _patched(argv, **kw):
                argv = [
                    a.replace("--enable-ldw-opt=false", "--enable-ldw-opt=true")
                    for a in argv
                ]
                return _orig(argv, **kw)

            _bu.run_command = _patched
            _bu._ldwopt_patched = True

    trace = bool(int(os.environ.get("BASS_TRACE", "0") or "0"))
    if trace:
        trace = _ensure_ntff_hook()
    res = run_bass_kernel_spmd(
        nc,
        in_maps,
        core_ids=list(range(NCORES)),
        trace=trace,
    )
    LAST_RESULT = res
    return _assemble([r["y"] for r in res.results])


# revision 70
# speedup vs baseline: 1.0134x; 1.0134x over previous
"""CTRGC graph-conv kernel for Trainium2, 8-core data-parallel.

Math (attention branch of the reference is dead code, all BNs fold away):
    y[n,o,t,v] = relu( sum_k sum_w Amix_k[w,v] * H_k[n,o,t,w] + B[o] )
    H_k[n,o,t,w] = sum_c W2_k[o,c] * x[n,c,t,w]

Device plan per core (n = 16 local batches):
  Stage A (conv): x is the *stationary* matmul operand: for each chunk of
    126 = 7t*18w positions, out[(t,w), (k,o)] = x_chunk[c,128].T @ W[c,192].
    This lands H transposed -- (t,w) on partitions -- which is what stage B
    needs.  Chunks 0..17 use x half-1 on partitions 0:64, chunks 18..36 use
    x half-2 on partitions 64:128, so pairs of matmuls run concurrently on
    disjoint PE row-groups.  lhsT slices are 128 wide -- the 2 junk columns
    only pollute out partitions 126/127 which are never read.
  Stage B (mix): stationary blockdiag(Amix_k x7) padded to [128,128] --
    32-aligned weight column counts take a ~2x faster LDWEIGHTS path (95ns
    vs 177-230ns measured for 126 cols); rhs = H slices [128, 8 chunks x
    64]; PSUM-accumulate over k.  Row 126 of H holds the bias pattern (B[o]
    at k=0 column slots, DMA'd per n) and row 126 of blkA_0 is ones => the
    matmul adds B[o] along the free dim for free; row 127 is zero in both
    (zero, not stale SBUF, so 0*0 cannot make NaNs).
  Epilogue: relu split across ScalarE/VectorE, fp16 out (host upcasts).

All device I/O is fp16 (host casts x down and y back up; PE accumulates in
fp32, so the extra error is ~2e-4).  Software-pipelined by one n: mix(n-1)
is emitted after conv(n) so its H inputs are already evacuated when the PE
reaches it.  Strict DVE/ACT alternation of PSUM-evac ops is load-bearing:
consecutive same-engine evacs stall PSUM bank recycling.

Chunking: t-groups of 7; chunk 36 covers t=249..255 (recomputes t=249..251,
host discards the duplicates).  Device output y_dev[n, (tl,v)=126,
(ch,o)=2368]; host reassembles to (n,o,t,v).

Measured on 8x trn2 (axon): HW exec ~145-146 us, rel err (absmax) ~5.7e-4.
Engine balance at final: PE ~116 us matmul union (conv LDW-path-bound at
~94ns/chunk-load; mix stream+drain-bound), DVE/ACT ~100 us each (PSUM evac
at 1 elem/cycle/lane), plus ~14 us head + ~15 us tail of Tile barriers/DMA.
Keep ALL matmul weight-operand column counts 32-aligned (conv lhsT 128,
blkA 128) -- misaligned counts fall off the fast LDWEIGHTS path.
"""

import os
import sys
import types

import numpy as np

import concourse.bass as bass
import concourse.mybir as mybir
import concourse.tile as tile
from concourse import bacc
from concourse.bass_utils import run_bass_kernel_spmd

EPS = 1e-5
N, CIN, T, V = 128, 64, 256, 18
NK = 3
CS = 64  # per-subset output channels
NCORES = 8
NLOC = N // NCORES  # 16
TW = T * V  # 4608
TG = 7  # t positions per chunk
PCH = TG * V  # 126 partitions per chunk
NCH = 37  # 36 aligned chunks + 1 overlap chunk (t0=249)
H1W = 2268  # x half-1 cols: t 0..125   (chunks 0..17,  partitions 0:64)
H2W = 2340  # x half-2 cols: t 126..255 (chunks 18..36, partitions 64:128)
XW = 2344  # x tile width (pad so 128-wide lhsT slices stay in bounds)
KO = NK * CS  # 192
HW = NCH * KO  # 7104 H columns
GRP = 8  # chunks per mix psum group (8*64 = 512 = one PSUM bank)
NGRP = 5  # groups of sizes 8,8,8,8,5
YW = NCH * CS  # 2368 y columns
F16 = mybir.dt.float16
F32 = mybir.dt.float32

LAST_RESULT = None  # BassKernelResults of the most recent run (for test.py)


def _ensure_ntff_hook() -> bool:
    """Register the axon NTFF profile hook if the image's antenv lacks it.

    Dev-only (used when BASS_TRACE=1); returns True if tracing can work.
    """
    try:
        from antenv.axon_hooks import get_axon_ntff_profile_hook

        return get_axon_ntff_profile_hook() is not None
    except ImportError:
        pass
    try:
        import antenv
        from trn_agent_boot.trn_boot import _ntff_profile_via_ctypes

        hook = _ntff_profile_via_ctypes("/opt/axon/libaxon_pjrt.so")
        if hook is None:
            return False
        mod = types.ModuleType("antenv.axon_hooks")
        holder = [hook]
        mod.set_axon_ntff_profile_hook = lambda h: holder.__setitem__(0, h)
        mod.get_axon_ntff_profile_hook = lambda: holder[0]
        sys.modules["antenv.axon_hooks"] = mod
        antenv.axon_hooks = mod
        return True
    except Exception as e:  # pragma: no cover - dev path
        print(f"ntff hook setup failed: {e}", file=sys.stderr)
        return False


def _chunk_col(ch: int) -> int:
    """Column offset of chunk ch inside its x half."""
    if ch < 18:
        return 126 * ch
    if ch < 36:
        return 126 * (ch - 18)
    return 2214  # t0 = 249


def _fold_weights(A, PA, Wta, bta, g_ta, b_ta, m_ta, v_ta, g_bn, b_bn, m_bn, v_bn):
    s1 = g_ta / np.sqrt(v_ta + EPS)  # (3, 64)
    s2 = g_bn / np.sqrt(v_bn + EPS)  # (64,)
    W2 = Wta * s1[:, :, None] * s2[None, :, None]  # (3, 64, 64) [k, o, c]
    b1 = (bta - m_ta) * s1 + b_ta  # (3, 64)
    B = (b1.sum(0) - m_bn) * s2 + b_bn  # (64,)
    Amix = A + PA  # (3, 18, 18)

    # wt [128, 192]: rows = c (duplicated on 64:128 for PE row-group 1),
    # cols = (k, o)
    w_koc = np.ascontiguousarray(W2.transpose(2, 0, 1)).reshape(64, KO)
    wt = np.zeros((128, KO), np.float16)
    wt[0:64] = w_koc
    wt[64:128] = w_koc

    # blk [128, 3*128]: per k a blockdiag of 7 copies of Amix[k] padded to
    # 128 cols (32-aligned weight loads take the fast LDW path); row 126 of
    # the k=0 slice is ones (bias contraction row); row 127 zero (K=128
    # alignment pad, paired with the zeroed H row 127).
    blk = np.zeros((128, NK * 128), np.float16)
    for k in range(NK):
        for j in range(TG):
            blk[18 * j : 18 * j + 18, 128 * k + 18 * j : 128 * k + 18 * j + 18] = Amix[
                k
            ]
    blk[126, 0:126] = 1.0

    # brow [2, 7104]: row 0 = B[o] at the k=0 column slot of every chunk;
    # row 1 = zeros (lands in H row 127 so the K=128 pad row is defined).
    brow = np.zeros((2, HW), np.float16)
    for ch in range(NCH):
        brow[0, KO * ch : KO * ch + CS] = B
    return wt, blk, brow


def _build_bass() -> bass.Bass:
    nc = bacc.Bacc()
    x_d = nc.dram_tensor("x", [NLOC, CIN, TW], F16, kind="ExternalInput")
    wt_d = nc.dram_tensor("wt", [128, KO], F16, kind="ExternalInput")
    blk_d = nc.dram_tensor("blk", [128, NK * 128], F16, kind="ExternalInput")
    brow_d = nc.dram_tensor("brow", [2, HW], F16, kind="ExternalInput")
    y_d = nc.dram_tensor("y", [NLOC, PCH, YW], F16, kind="ExternalOutput")

    # conv psum tiles: [128, 1024] = 2 banks = four chunks at col offsets
    # {0, 192, 512, 704}
    rg0_tiles = [tuple(range(4 * i, 4 * i + 4)) for i in range(4)] + [(16, 17)]
    rg1_tiles = [tuple(range(18 + 4 * i, 22 + 4 * i)) for i in range(4)] + [
        (34, 35),
        (36,),
    ]

    with tile.TileContext(nc) as tc:
        with (
            tc.tile_pool(name="consts", bufs=1) as consts,
            tc.tile_pool(name="xp", bufs=6) as xpool,
            tc.tile_pool(name="hp", bufs=4) as hpool,
            tc.tile_pool(name="yp", bufs=3) as ypool,
            tc.tile_pool(name="cps", bufs=3, space="PSUM") as cpsum,
            tc.tile_pool(name="yps", bufs=2, space="PSUM") as ypsum,
        ):
            ei = 0  # evac engine alternator

            def load_x(n, first=False):
                xt = xpool.tile([128, XW], F16, name="xt", tag="xt")
                if first:
                    # split at the chunk-9 boundary: chunks 0-8 (cols 0:1134)
                    # become ready one DMA earlier (deps are region-level)
                    nc.sync.dma_start(out=xt[0:64, 0:1134], in_=x_d[n, :, 0:1134])
                    nc.sync.dma_start(
                        out=xt[0:64, 1134:H1W], in_=x_d[n, :, 1134:H1W]
                    )
                else:
                    nc.sync.dma_start(out=xt[0:64, 0:H1W], in_=x_d[n, :, 0:H1W])
                nc.sync.dma_start(out=xt[64:128, 0:H2W], in_=x_d[n, :, H1W:TW])
                return xt

            def emit_conv(n, xt):
                nonlocal ei
                ht = hpool.tile([128, HW], F16, name="ht", tag="ht")
                nc.gpsimd.dma_start(out=ht[126:128, :], in_=brow_d[:])

                # row-group-paired matmuls
                for ti in range(6):
                    pts = []
                    for rg, tiles in ((0, rg0_tiles), (1, rg1_tiles)):
                        if ti >= len(tiles):
                            pts.append(None)
                            continue
                        pts.append(
                            (
                                cpsum.tile([128, 1024], F32, name="cpt", tag="cpt"),
                                tiles[ti],
                                rg,
                            )
                        )
                    for j in range(4):
                        for ent in pts:
                            if ent is None or j >= len(ent[1]):
                                continue
                            pt, chunks, rg = ent
                            ch = chunks[j]
                            r0 = 64 * rg
                            c0 = _chunk_col(ch)
                            pcol = 512 * (j // 2) + 192 * (j % 2)
                            nc.tensor.matmul(
                                out=pt[:, pcol : pcol + 192],
                                lhsT=xt[r0 : r0 + 64, c0 : c0 + 128],
                                rhs=wt_t[r0 : r0 + 64, :],
                                start=True,
                                stop=True,
                            )
                    for ent in pts:
                        if ent is None:
                            continue
                        pt, chunks, rg = ent
                        nch_t = len(chunks)
                        nb = (nch_t + 1) // 2  # banks used
                        fw = 192 * min(nch_t, 2)
                        src = bass.AP(
                            tensor=pt.tensor,
                            offset=pt.offset,
                            ap=[[pt.ap[0][0], 126], [512, nb], [1, fw]],
                        )
                        dst = bass.AP(
                            tensor=ht.tensor,
                            offset=ht.offset + KO * chunks[0],
                            ap=[[ht.ap[0][0], 126], [fw, nb], [1, fw]],
                        )
                        if ei % 2 == 0:
                            nc.vector.tensor_copy(out=dst, in_=src)
                        else:
                            nc.scalar.copy(out=dst, in_=src)
                        ei += 1
                return ht

            def emit_mix(n, ht):
                # k-outer over group-halves: same blkA_k stationary across the
                # groups of a half -> no weight reload between matmuls, and
                # consecutive matmuls hit different PSUM banks (pipelined).
                nonlocal ei
                yt = ypool.tile([128, YW], F16, name="yt", tag="yt")
                hv = ht[0:128].rearrange("p (c f) -> p c f", f=KO)
                for g in range(NGRP):
                    wg = GRP if g < 4 else NCH - 4 * GRP
                    fw = wg * CS
                    pt = ypsum.tile([128, 512], F32, name="ypt", tag="ypt")
                    out = pt[:, 0:fw].rearrange("p (c o) -> p c o", o=CS)
                    for k in range(NK):
                        rhs = hv[:, GRP * g : GRP * g + wg, CS * k : CS * k + CS]
                        nc.tensor.matmul(
                            out=out,
                            lhsT=blk_t[:, 128 * k : 128 * k + 128],
                            rhs=rhs,
                            start=(k == 0),
                            stop=(k == NK - 1),
                        )
                    # split relu across both engines so the psum bank frees
                    # ~2x sooner (mix is gated on psum recycling)
                    h1 = fw // 2
                    nc.vector.tensor_scalar_max(
                        yt[0:126, 512 * g : 512 * g + h1], pt[0:126, 0:h1], 0.0
                    )
                    nc.scalar.activation(
                        yt[0:126, 512 * g + h1 : 512 * g + fw],
                        pt[0:126, h1:fw],
                        mybir.ActivationFunctionType.Relu,
                    )
                if n == NLOC - 1:
                    # tail: split the last store across both idle queues
                    nc.gpsimd.dma_start(out=y_d[n, :, 0:1024], in_=yt[0:126, 0:1024])
                    nc.sync.dma_start(out=y_d[n, :, 1024:YW], in_=yt[0:126, 1024:YW])
                else:
                    nc.gpsimd.dma_start(out=y_d[n], in_=yt[0:126, :])

            # software pipeline by one n: mix(n-1) is emitted after conv(n),
            # so its H inputs are fully evacuated by the time PE reaches it.
            x0 = load_x(0, first=True)  # x(0) ahead of consts: it's the longer pole
            wt_t = consts.tile([128, KO], F16)
            nc.sync.dma_start(out=wt_t[:], in_=wt_d[:])
            blk_t = consts.tile([128, NK * 128], F16)
            nc.sync.dma_start(out=blk_t[:], in_=blk_d[:])

            prev = None
            xt_next = x0
            for n in range(NLOC):
                xt = xt_next if xt_next is not None else load_x(n)
                ht = emit_conv(n, xt)
                xt_next = load_x(n + 1) if n + 1 < NLOC else None
                if prev is not None:
                    emit_mix(n - 1, prev)
                prev = ht
            emit_mix(NLOC - 1, prev)
    nc.compile()
    return nc


def _assemble(parts: list[np.ndarray]) -> np.ndarray:
    """parts: NCORES arrays [NLOC, 126, 2368] -> full (N, 64, 256, 18)."""
    y = np.empty((N, CS, T, V), np.float32)
    for i, p in enumerate(parts):
        r = p.astype(np.float32).reshape(NLOC, TG, V, NCH, CS)  # [n, tl, v, ch, o]
        main = r[:, :, :, 0:36, :].transpose(0, 4, 3, 1, 2).reshape(NLOC, CS, 252, V)
        tail = r[:, 3:7, :, 36, :].transpose(0, 3, 1, 2)  # t = 252..255
        sl = slice(i * NLOC, (i + 1) * NLOC)
        y[sl, :, 0:252, :] = main
        y[sl, :, 252:256, :] = tail
    return y


def kernel(
    x,
    A,
    PA,
    Wta,
    bta,
    g_ta,
    b_ta,
    m_ta,
    v_ta,
    Wsa=None,
    bsa=None,
    Wsb=None,
    bsb=None,
    g_bn=None,
    b_bn=None,
    m_bn=None,
    v_bn=None,
):
    global LAST_RESULT
    f = np.asarray
    wt, blk, brow = _fold_weights(
        f(A, dtype=np.float32),
        f(PA, dtype=np.float32),
        f(Wta, dtype=np.float32),
        f(bta, dtype=np.float32),
        f(g_ta, dtype=np.float32),
        f(b_ta, dtype=np.float32),
        f(m_ta, dtype=np.float32),
        f(v_ta, dtype=np.float32),
        f(g_bn, dtype=np.float32),
        f(b_bn, dtype=np.float32),
        f(m_bn, dtype=np.float32),
        f(v_bn, dtype=np.float32),
    )
    xsh = (
        np.asarray(x, dtype=np.float32)
        .reshape(NCORES, NLOC, CIN, TW)
        .astype(np.float16)
    )
    nc = _build_bass()
    in_maps = [
        {"x": np.ascontiguousarray(xsh[i]), "wt": wt, "blk": blk, "brow": brow}
        for i in range(NCORES)
    ]
    if os.environ.get("BASS_LDWOPT"):  # dev experiment: enable walrus ldw-opt
        import concourse.bass_utils as _bu

        if not getattr(_bu, "_ldwopt_patched", False):
            _orig = _bu.run_command

            def _patched(argv, **kw):
                argv = [
                    a.replace("--enable-ldw-opt=false", "--enable-ldw-opt=true")
                    for a in argv
                ]
                return _orig(argv, **kw)

            _bu.run_command = _patched
            _bu._ldwopt_patched = True

    trace = bool(int(os.environ.get("BASS_TRACE", "0") or "0"))
    if trace:
        trace = _ensure_ntff_hook()
    res = run_bass_kernel_spmd(
        nc,
        in_maps,
        core_ids=list(range(NCORES)),
        trace=trace,
    )
    LAST_RESULT = res
    return _assemble([r["y"] for r in res.results])


# revision 71
# speedup vs baseline: 1.0145x; 1.0011x over previous
"""CTRGC graph-conv kernel for Trainium2, 8-core data-parallel.

Math (attention branch of the reference is dead code, all BNs fold away):
    y[n,o,t,v] = relu( sum_k sum_w Amix_k[w,v] * H_k[n,o,t,w] + B[o] )
    H_k[n,o,t,w] = sum_c W2_k[o,c] * x[n,c,t,w]

Device plan per core (n = 16 local batches):
  Stage A (conv): x is the *stationary* matmul operand: for each chunk of
    126 = 7t*18w positions, out[(t,w), (k,o)] = x_chunk[c,128].T @ W[c,192].
    This lands H transposed -- (t,w) on partitions -- which is what stage B
    needs.  Chunks 0..17 use x half-1 on partitions 0:64, chunks 18..36 use
    x half-2 on partitions 64:128, so pairs of matmuls run concurrently on
    disjoint PE row-groups.  lhsT slices are 128 wide -- the 2 junk columns
    only pollute out partitions 126/127 which are never read.
  Stage B (mix): stationary blockdiag(Amix_k x7) padded to [128,128] --
    32-aligned weight column counts take a ~2x faster LDWEIGHTS path (95ns
    vs 177-230ns measured for 126 cols); rhs = H slices [128, 8 chunks x
    64]; PSUM-accumulate over k.  Row 126 of H holds the bias pattern (B[o]
    at k=0 column slots, DMA'd per n) and row 126 of blkA_0 is ones => the
    matmul adds B[o] along the free dim for free; row 127 is zero in both
    (zero, not stale SBUF, so 0*0 cannot make NaNs).
  Epilogue: relu split across ScalarE/VectorE, fp16 out (host upcasts).

All device I/O is fp16 (host casts x down and y back up; PE accumulates in
fp32, so the extra error is ~2e-4).  Software-pipelined by one n: mix(n-1)
is emitted after conv(n) so its H inputs are already evacuated when the PE
reaches it.  Strict DVE/ACT alternation of PSUM-evac ops is load-bearing:
consecutive same-engine evacs stall PSUM bank recycling.

Chunking: t-groups of 7; chunk 36 covers t=249..255 (recomputes t=249..251,
host discards the duplicates).  Device output y_dev[n, (tl,v)=126,
(ch,o)=2368]; host reassembles to (n,o,t,v).

Measured on 8x trn2 (axon): HW exec ~145-146 us, rel err (absmax) ~5.7e-4.
Engine balance at final: PE ~116 us matmul union (conv LDW-path-bound at
~94ns/chunk-load; mix stream+drain-bound), DVE/ACT ~100 us each (PSUM evac
at 1 elem/cycle/lane), plus ~14 us head + ~15 us tail of Tile barriers/DMA.
Keep ALL matmul weight-operand column counts 32-aligned (conv lhsT 128,
blkA 128) -- misaligned counts fall off the fast LDWEIGHTS path.
"""

import os
import sys
import types

import numpy as np

import concourse.bass as bass
import concourse.mybir as mybir
import concourse.tile as tile
from concourse import bacc
from concourse.bass_utils import run_bass_kernel_spmd

EPS = 1e-5
N, CIN, T, V = 128, 64, 256, 18
NK = 3
CS = 64  # per-subset output channels
NCORES = 8
NLOC = N // NCORES  # 16
TW = T * V  # 4608
TG = 7  # t positions per chunk
PCH = TG * V  # 126 partitions per chunk
NCH = 37  # 36 aligned chunks + 1 overlap chunk (t0=249)
H1W = 2268  # x half-1 cols: t 0..125   (chunks 0..17,  partitions 0:64)
H2W = 2340  # x half-2 cols: t 126..255 (chunks 18..36, partitions 64:128)
XW = 2344  # x tile width (pad so 128-wide lhsT slices stay in bounds)
KO = NK * CS  # 192
HW = NCH * KO  # 7104 H columns
GRP = 8  # chunks per mix psum group (8*64 = 512 = one PSUM bank)
NGRP = 5  # groups of sizes 8,8,8,8,5
YW = NCH * CS  # 2368 y columns
F16 = mybir.dt.float16
F32 = mybir.dt.float32

LAST_RESULT = None  # BassKernelResults of the most recent run (for test.py)


def _ensure_ntff_hook() -> bool:
    """Register the axon NTFF profile hook if the image's antenv lacks it.

    Dev-only (used when BASS_TRACE=1); returns True if tracing can work.
    """
    try:
        from antenv.axon_hooks import get_axon_ntff_profile_hook

        return get_axon_ntff_profile_hook() is not None
    except ImportError:
        pass
    try:
        import antenv
        from trn_agent_boot.trn_boot import _ntff_profile_via_ctypes

        hook = _ntff_profile_via_ctypes("/opt/axon/libaxon_pjrt.so")
        if hook is None:
            return False
        mod = types.ModuleType("antenv.axon_hooks")
        holder = [hook]
        mod.set_axon_ntff_profile_hook = lambda h: holder.__setitem__(0, h)
        mod.get_axon_ntff_profile_hook = lambda: holder[0]
        sys.modules["antenv.axon_hooks"] = mod
        antenv.axon_hooks = mod
        return True
    except Exception as e:  # pragma: no cover - dev path
        print(f"ntff hook setup failed: {e}", file=sys.stderr)
        return False


def _chunk_col(ch: int) -> int:
    """Column offset of chunk ch inside its x half."""
    if ch < 18:
        return 126 * ch
    if ch < 36:
        return 126 * (ch - 18)
    return 2214  # t0 = 249


def _fold_weights(A, PA, Wta, bta, g_ta, b_ta, m_ta, v_ta, g_bn, b_bn, m_bn, v_bn):
    s1 = g_ta / np.sqrt(v_ta + EPS)  # (3, 64)
    s2 = g_bn / np.sqrt(v_bn + EPS)  # (64,)
    W2 = Wta * s1[:, :, None] * s2[None, :, None]  # (3, 64, 64) [k, o, c]
    b1 = (bta - m_ta) * s1 + b_ta  # (3, 64)
    B = (b1.sum(0) - m_bn) * s2 + b_bn  # (64,)
    Amix = A + PA  # (3, 18, 18)

    # wt [128, 192]: rows = c (duplicated on 64:128 for PE row-group 1),
    # cols = (k, o)
    w_koc = np.ascontiguousarray(W2.transpose(2, 0, 1)).reshape(64, KO)
    wt = np.zeros((128, KO), np.float16)
    wt[0:64] = w_koc
    wt[64:128] = w_koc

    # blk [128, 3*128]: per k a blockdiag of 7 copies of Amix[k] padded to
    # 128 cols (32-aligned weight loads take the fast LDW path); row 126 of
    # the k=0 slice is ones (bias contraction row); row 127 zero (K=128
    # alignment pad, paired with the zeroed H row 127).
    blk = np.zeros((128, NK * 128), np.float16)
    for k in range(NK):
        for j in range(TG):
            blk[18 * j : 18 * j + 18, 128 * k + 18 * j : 128 * k + 18 * j + 18] = Amix[
                k
            ]
    blk[126, 0:126] = 1.0

    # brow [2, 7104]: row 0 = B[o] at the k=0 column slot of every chunk;
    # row 1 = zeros (lands in H row 127 so the K=128 pad row is defined).
    brow = np.zeros((2, HW), np.float16)
    for ch in range(NCH):
        brow[0, KO * ch : KO * ch + CS] = B
    return wt, blk, brow


def _build_bass() -> bass.Bass:
    nc = bacc.Bacc()
    x_d = nc.dram_tensor("x", [NLOC, CIN, TW], F16, kind="ExternalInput")
    wt_d = nc.dram_tensor("wt", [128, KO], F16, kind="ExternalInput")
    blk_d = nc.dram_tensor("blk", [128, NK * 128], F16, kind="ExternalInput")
    brow_d = nc.dram_tensor("brow", [2, HW], F16, kind="ExternalInput")
    y_d = nc.dram_tensor("y", [NLOC, PCH, YW], F16, kind="ExternalOutput")

    # conv psum tiles: [128, 1024] = 2 banks = four chunks at col offsets
    # {0, 192, 512, 704}
    rg0_tiles = [tuple(range(4 * i, 4 * i + 4)) for i in range(4)] + [(16, 17)]
    rg1_tiles = [tuple(range(18 + 4 * i, 22 + 4 * i)) for i in range(4)] + [
        (34, 35),
        (36,),
    ]

    with tile.TileContext(nc) as tc:
        with (
            tc.tile_pool(name="consts", bufs=1) as consts,
            tc.tile_pool(name="xp", bufs=6) as xpool,
            tc.tile_pool(name="hp", bufs=4) as hpool,
            tc.tile_pool(name="yp", bufs=3) as ypool,
            tc.tile_pool(name="cps", bufs=3, space="PSUM") as cpsum,
            tc.tile_pool(name="yps", bufs=2, space="PSUM") as ypsum,
        ):
            ei = 0  # evac engine alternator

            def load_x(n, first=False):
                xt = xpool.tile([128, XW], F16, name="xt", tag="xt")
                if first:
                    # split both halves at their chunk-9 boundaries and load
                    # the leading pieces first: the first conv row-group PAIR
                    # (chunks 0 and 18) is ready after two small DMAs
                    # (deps are region-level)
                    nc.sync.dma_start(out=xt[0:64, 0:1134], in_=x_d[n, :, 0:1134])
                    nc.sync.dma_start(
                        out=xt[64:128, 0:1134], in_=x_d[n, :, H1W : H1W + 1134]
                    )
                    nc.sync.dma_start(
                        out=xt[0:64, 1134:H1W], in_=x_d[n, :, 1134:H1W]
                    )
                    nc.sync.dma_start(
                        out=xt[64:128, 1134:H2W], in_=x_d[n, :, H1W + 1134 : TW]
                    )
                else:
                    nc.sync.dma_start(out=xt[0:64, 0:H1W], in_=x_d[n, :, 0:H1W])
                    nc.sync.dma_start(
                        out=xt[64:128, 0:H2W], in_=x_d[n, :, H1W:TW]
                    )
                return xt

            def emit_conv(n, xt):
                nonlocal ei
                ht = hpool.tile([128, HW], F16, name="ht", tag="ht")
                nc.gpsimd.dma_start(out=ht[126:128, :], in_=brow_d[:])

                # row-group-paired matmuls
                for ti in range(6):
                    pts = []
                    for rg, tiles in ((0, rg0_tiles), (1, rg1_tiles)):
                        if ti >= len(tiles):
                            pts.append(None)
                            continue
                        pts.append(
                            (
                                cpsum.tile([128, 1024], F32, name="cpt", tag="cpt"),
                                tiles[ti],
                                rg,
                            )
                        )
                    for j in range(4):
                        for ent in pts:
                            if ent is None or j >= len(ent[1]):
                                continue
                            pt, chunks, rg = ent
                            ch = chunks[j]
                            r0 = 64 * rg
                            c0 = _chunk_col(ch)
                            pcol = 512 * (j // 2) + 192 * (j % 2)
                            nc.tensor.matmul(
                                out=pt[:, pcol : pcol + 192],
                                lhsT=xt[r0 : r0 + 64, c0 : c0 + 128],
                                rhs=wt_t[r0 : r0 + 64, :],
                                start=True,
                                stop=True,
                            )
                    for ent in pts:
                        if ent is None:
                            continue
                        pt, chunks, rg = ent
                        nch_t = len(chunks)
                        nb = (nch_t + 1) // 2  # banks used
                        fw = 192 * min(nch_t, 2)
                        src = bass.AP(
                            tensor=pt.tensor,
                            offset=pt.offset,
                            ap=[[pt.ap[0][0], 126], [512, nb], [1, fw]],
                        )
                        dst = bass.AP(
                            tensor=ht.tensor,
                            offset=ht.offset + KO * chunks[0],
                            ap=[[ht.ap[0][0], 126], [fw, nb], [1, fw]],
                        )
                        if ei % 2 == 0:
                            nc.vector.tensor_copy(out=dst, in_=src)
                        else:
                            nc.scalar.copy(out=dst, in_=src)
                        ei += 1
                return ht

            def emit_mix(n, ht):
                # k-outer over group-halves: same blkA_k stationary across the
                # groups of a half -> no weight reload between matmuls, and
                # consecutive matmuls hit different PSUM banks (pipelined).
                nonlocal ei
                yt = ypool.tile([128, YW], F16, name="yt", tag="yt")
                hv = ht[0:128].rearrange("p (c f) -> p c f", f=KO)
                for g in range(NGRP):
                    wg = GRP if g < 4 else NCH - 4 * GRP
                    fw = wg * CS
                    pt = ypsum.tile([128, 512], F32, name="ypt", tag="ypt")
                    out = pt[:, 0:fw].rearrange("p (c o) -> p c o", o=CS)
                    for k in range(NK):
                        rhs = hv[:, GRP * g : GRP * g + wg, CS * k : CS * k + CS]
                        nc.tensor.matmul(
                            out=out,
                            lhsT=blk_t[:, 128 * k : 128 * k + 128],
                            rhs=rhs,
                            start=(k == 0),
                            stop=(k == NK - 1),
                        )
                    # split relu across both engines so the psum bank frees
                    # ~2x sooner (mix is gated on psum recycling)
                    h1 = fw // 2
                    nc.vector.tensor_scalar_max(
                        yt[0:126, 512 * g : 512 * g + h1], pt[0:126, 0:h1], 0.0
                    )
                    nc.scalar.activation(
                        yt[0:126, 512 * g + h1 : 512 * g + fw],
                        pt[0:126, h1:fw],
                        mybir.ActivationFunctionType.Relu,
                    )
                if n == NLOC - 1:
                    # tail: split the last store across both idle queues
                    nc.gpsimd.dma_start(out=y_d[n, :, 0:1024], in_=yt[0:126, 0:1024])
                    nc.sync.dma_start(out=y_d[n, :, 1024:YW], in_=yt[0:126, 1024:YW])
                else:
                    nc.gpsimd.dma_start(out=y_d[n], in_=yt[0:126, :])

            # software pipeline by one n: mix(n-1) is emitted after conv(n),
            # so its H inputs are fully evacuated by the time PE reaches it.
            x0 = load_x(0, first=True)  # x(0) ahead of consts: it's the longer pole
            wt_t = consts.tile([128, KO], F16)
            nc.sync.dma_start(out=wt_t[:], in_=wt_d[:])
            blk_t = consts.tile([128, NK * 128], F16)
            nc.sync.dma_start(out=blk_t[:], in_=blk_d[:])

            prev = None
            xt_next = x0
            for n in range(NLOC):
                xt = xt_next if xt_next is not None else load_x(n)
                ht = emit_conv(n, xt)
                xt_next = load_x(n + 1) if n + 1 < NLOC else None
                if prev is not None:
                    emit_mix(n - 1, prev)
                prev = ht
            emit_mix(NLOC - 1, prev)
    nc.compile()
    return nc


def _assemble(parts: list[np.ndarray]) -> np.ndarray:
    """parts: NCORES arrays [NLOC, 126, 2368] -> full (N, 64, 256, 18)."""
    y = np.empty((N, CS, T, V), np.float32)
    for i, p in enumerate(parts):
        r = p.astype(np.float32).reshape(NLOC, TG, V, NCH, CS)  # [n, tl, v, ch, o]
        main = r[:, :, :, 0:36, :].transpose(0, 4, 3, 1, 2).reshape(NLOC, CS, 252, V)
        tail = r[:, 3:7, :, 36, :].transpose(0, 3, 1, 2)  # t = 252..255
        sl = slice(i * NLOC, (i + 1) * NLOC)
        y[sl, :, 0:252, :] = main
        y[sl, :, 252:256, :] = tail
    return y


def kernel(
    x,
    A,
    PA,
    Wta,
    bta,
    g_ta,
    b_ta,
    m_ta,
    v_ta,
    Wsa=None,
    bsa=None,
    Wsb=None,
    bsb=None,
    g_bn=None,
    b_bn=None,
    m_bn=None,
    v_bn=None,
):
    global LAST_RESULT
    f = np.asarray
    wt, blk, brow = _fold_weights(
        f(A, dtype=np.float32),
        f(PA, dtype=np.float32),
        f(Wta, dtype=np.float32),
        f(bta, dtype=np.float32),
        f(g_ta, dtype=np.float32),
        f(b_ta, dtype=np.float32),
        f(m_ta, dtype=np.float32),
        f(v_ta, dtype=np.float32),
        f(g_bn, dtype=np.float32),
        f(b_bn, dtype=np.float32),
        f(m_bn, dtype=np.float32),
        f(v_bn, dtype=np.float32),
    )
    xsh = (
        np.asarray(x, dtype=np.float32)
        .reshape(NCORES, NLOC, CIN, TW)
        .astype(np.float16)
    )
    nc = _build_bass()
    in_maps = [
        {"x": np.ascontiguousarray(xsh[i]), "wt": wt, "blk": blk, "brow": brow}
        for i in range(NCORES)
    ]
    if os.environ.get("BASS_LDWOPT"):  # dev experiment: enable walrus ldw-opt
        import concourse.bass_utils as _bu

        if not getattr(_bu, "_ldwopt_patched", False):
            _orig = _bu.run_command

            def _patched(argv, **kw):
                argv = [
                    a.replace("--enable-ldw-opt=false", "--enable-ldw-opt=true")
                    for a in argv
                ]
                return _orig(argv, **kw)

            _bu.run_command = _patched
            _bu._ldwopt_patched = True

    trace = bool(int(os.environ.get("BASS_TRACE", "0") or "0"))
    if trace:
        trace = _ensure_ntff_hook()
    res = run_bass_kernel_spmd(
        nc,
        in_maps,
        core_ids=list(range(NCORES)),
        trace=trace,
    )
    LAST_RESULT = res
    return _assemble([r["y"] for r in res.results])
